# revision 4
# baseline (speedup 1.0000x reference)
"""Causal self-attention (B=2, S=2048, E=1024, H=16) on 8 TRN2 NeuronCores.

Sharding: head-parallel. Core c owns heads {2c, 2c+1} (128 of the 1024
hidden dims). Each core reads the full X, computes Q/K/V for its heads,
causal-softmax attention, and a partial output projection through its
slice of Wp's columns; the host sums the 8 partials and adds the bias.

On-chip layout is fully "transposed" so no operand ever needs an X/Q/K
transpose on device:
  - X is pre-transposed on host to XT [E, B*S].
  - QKV matmuls produce qT/kT/vT [dims, tokens] directly.
  - Scores are computed transposed, sT[k, q] = kT.T-tile @ qT-tile, so the
    exp'd scores feed the AV matmul as the moving operand.
  - V is re-transposed to natural [token, dim] via PE transposes, extended
    with a ones column so the AV matmul also yields the softmax denominators.
  - Softmax uses no running-max: with this problem's N(0,1)-scaled inputs,
    scores are O(10), far inside fp32 exp range.
  - All large matmuls run in float32r (TF32-like, ~1e-4 rel err, full PE rate).
"""
import sys

for _p in ("/opt/trn_rl_repo", "/root/.axon_site/_ro/trn_rl_repo"):
    if _p not in sys.path:
        sys.path.append(_p)

import numpy as np
import concourse.bacc as bacc
import concourse.mybir as mybir
from concourse import tile, masks
from concourse import bass_utils

N_CORES = 8
B, S, E, H = 2, 2048, 1024, 16
DK = E // H          # 64
HPC = H // N_CORES   # 2 heads per core
LD = HPC * DK        # 128 local dims per core
TOK = B * S          # 4096 tokens
NEG = -1.0e30

F32 = mybir.dt.float32
F32R = mybir.dt.float32r

_NC_CACHE = {}


def build_nc():
    nc = bacc.Bacc("TRN2", target_bir_lowering=False, debug=False,
                   num_devices=N_CORES)
    xt_d = nc.dram_tensor("xt", [8, 128, TOK], F32R, kind="ExternalInput").ap()
    wqt_d = nc.dram_tensor("wqt", [8, 128, LD], F32R, kind="ExternalInput").ap()
    wkt_d = nc.dram_tensor("wkt", [8, 128, LD], F32R, kind="ExternalInput").ap()
    wvt_d = nc.dram_tensor("wvt", [8, 128, LD], F32R, kind="ExternalInput").ap()
    wpt_d = nc.dram_tensor("wpt", [128, E], F32R, kind="ExternalInput").ap()
    mask_d = nc.dram_tensor("mask", [128, 128], F32, kind="ExternalInput").ap()
    yp_d = nc.dram_tensor("ypart", [TOK, E], F32, kind="ExternalOutput").ap()

    Exp = mybir.ActivationFunctionType.Exp
    NQC = S // 512            # q-chunks per sequence (4)
    NKT = S // 128            # k-tiles per sequence (16)

    with tile.TileContext(nc) as tc:
        with (
            tc.tile_pool(name="const", bufs=1) as cpool,
            tc.tile_pool(name="wgt", bufs=1) as wpool,
            tc.tile_pool(name="qkv", bufs=1) as qkvpool,
            tc.tile_pool(name="xe", bufs=4) as xpool,
            tc.tile_pool(name="pt", bufs=3) as ptpool,
            tc.tile_pool(name="small", bufs=2) as spool,
            tc.tile_pool(name="yn", bufs=2) as ynpool,
            tc.tile_pool(name="out", bufs=3) as opool,
        ):
            ident = cpool.tile([128, 128], F32, tag="ident")
            masks.make_identity(nc, ident[:])
            mask_sb = cpool.tile([128, 128], F32, tag="mask")
            nc.sync.dma_start(out=mask_sb[:], in_=mask_d)
            ones1 = cpool.tile([1, 64], F32R, tag="ones1")
            nc.gpsimd.memset(ones1[:].bitcast(F32), 1.0)

            wqt = wpool.tile([128, 8, LD], F32R, tag="wqt")
            wkt = wpool.tile([128, 8, LD], F32R, tag="wkt")
            wvt = wpool.tile([128, 8, LD], F32R, tag="wvt")
            wpt = wpool.tile([128, E], F32R, tag="wpt")
            nc.sync.dma_start(out=wqt[:], in_=wqt_d.rearrange("a p m -> p a m"))
            nc.sync.dma_start(out=wkt[:], in_=wkt_d.rearrange("a p m -> p a m"))
            nc.sync.dma_start(out=wvt[:], in_=wvt_d.rearrange("a p m -> p a m"))
            nc.sync.dma_start(out=wpt[:], in_=wpt_d)

            qT = qkvpool.tile([128, TOK], F32R, tag="qT")
            kT = qkvpool.tile([128, TOK], F32R, tag="kT")
            vTs = qkvpool.tile([128, TOK], F32, tag="vTs")
            # v_ext per head: [token-in-tile, k-tile, dim(+ones)]
            vext = [qkvpool.tile([128, TOK // 128, DK + 1], F32R,
                                 tag=f"vext{h}", name=f"vext{h}")
                    for h in range(HPC)]
            for h in range(HPC):
                nc.gpsimd.memset(vext[h][:].bitcast(F32), 1.0)

            # ---------- Phase 1: QKV projections + v_ext build ----------
            with tc.tile_pool(name="ps1", bufs=2, space="PSUM") as ps1:
                for j in range(TOK // 512):
                    jsl = slice(512 * j, 512 * (j + 1))
                    psq = ps1.tile([128, 512], F32, tag="psq")
                    psk = ps1.tile([128, 512], F32, tag="psk")
                    psv = ps1.tile([128, 512], F32, tag="psv")
                    for e in range(8):
                        xe = xpool.tile([128, 512], F32R, tag="xe")
                        nc.sync.dma_start(out=xe[:], in_=xt_d[e, :, jsl])
                        st, sp = (e == 0), (e == 7)
                        nc.tensor.matmul(psq[:], wqt[:, e], xe[:], start=st, stop=sp)
                        nc.tensor.matmul(psk[:], wkt[:, e], xe[:], start=st, stop=sp)
                        nc.tensor.matmul(psv[:], wvt[:, e], xe[:], start=st, stop=sp)
                    nc.scalar.copy(qT[:, jsl], psq[:])
                    nc.scalar.copy(kT[:, jsl], psk[:])
                    nc.vector.tensor_copy(vTs[:, jsl], psv[:])
                    for h in range(HPC):
                        hsl = slice(DK * h, DK * (h + 1))
                        for t in range(4):
                            kti = 4 * j + t
                            tp = ps1.tile([128, DK], F32, tag="tp")
                            nc.tensor.transpose(
                                tp[:], vTs[hsl, 128 * kti:128 * (kti + 1)],
                                ident[hsl, hsl])
                            nc.vector.tensor_copy(vext[h][:, kti, 0:DK], tp[:])

            # ---------- Phase 2: attention + partial projection ----------
            with tc.tile_pool(name="ps2", bufs=1, space="PSUM") as ps2:
                for b in range(B):
                    for qc in range(NQC):
                        q0 = S * b + 512 * qc
                        qsl = slice(q0, q0 + 512)
                        ynorm = ynpool.tile([128, 512], F32R, tag="yn")
                        for h in range(HPC):
                            hsl = slice(DK * h, DK * (h + 1))
                            nkt = 4 * (qc + 1)
                            yext = ps2.tile([DK + 1, 512], F32, tag="yext",
                                            bufs=2)
                            for kt in range(nkt):
                                kg = NKT * b + kt
                                ksl = slice(128 * kg, 128 * (kg + 1))
                                sps = ps2.tile([128, 512], F32, tag="sps",
                                               bufs=2)
                                nc.tensor.matmul(sps[:], kT[hsl, ksl],
                                                 qT[hsl, qsl],
                                                 start=True, stop=True)
                                pt = ptpool.tile([128, 512], F32R, tag="pt")
                                i = kt - 4 * qc
                                if i >= 0:
                                    r = 128 * i
                                    nc.vector.tensor_add(
                                        sps[:, r:r + 128], sps[:, r:r + 128],
                                        mask_sb[:])
                                    if r > 0:
                                        nc.gpsimd.memset(
                                            pt[:, 0:r].bitcast(F32), 0.0)
                                    nc.scalar.activation(pt[:, r:512],
                                                         sps[:, r:512], Exp)
                                else:
                                    nc.scalar.activation(pt[:], sps[:], Exp)
                                nc.tensor.matmul(yext[:], vext[h][:, kg],
                                                 pt[:], start=(kt == 0),
                                                 stop=(kt == nkt - 1))
                            ysb = spool.tile([DK, 512], F32, tag="ysb")
                            nc.scalar.copy(ysb[:], yext[0:DK, :])
                            rsb = spool.tile([1, 512], F32R, tag="rsb")
                            with nc.allow_low_precision(
                                    reason="f32r rounding of softmax recip"):
                                nc.vector.reciprocal(rsb[:], yext[DK:DK + 1, :])
                            rep = ps2.tile([DK, 512], F32, tag="rep")
                            nc.tensor.matmul(rep[:], ones1[:], rsb[:],
                                             start=True, stop=True)
                            nc.vector.tensor_mul(ynorm[hsl, :], ysb[:], rep[:])
                        for t in range(4):
                            tok0 = q0 + 128 * t
                            for eo in range(2):
                                esl = slice(512 * eo, 512 * (eo + 1))
                                ops = ps2.tile([128, 512], F32, tag="ops",
                                               bufs=2)
                                nc.tensor.matmul(ops[:],
                                                 ynorm[:, 128 * t:128 * (t + 1)],
                                                 wpt[:, esl],
                                                 start=True, stop=True)
                                osb = opool.tile([128, 512], F32, tag="osb")
                                nc.scalar.copy(osb[:], ops[:])
                                nc.sync.dma_start(
                                    out=yp_d[tok0:tok0 + 128, esl],
                                    in_=osb[:])

    nc.compile()
    return nc


def get_nc():
    if "nc" not in _NC_CACHE:
        _NC_CACHE["nc"] = build_nc()
    return _NC_CACHE["nc"]


def make_in_maps(X, Wq, Wk, Wv, Wp):
    X = np.ascontiguousarray(np.asarray(X, dtype=np.float32)).reshape(TOK, E)
    XT = np.ascontiguousarray(X.T).reshape(8, 128, TOK)
    scale = np.float32(1.0 / np.sqrt(DK))
    mask = np.where(np.arange(128)[:, None] <= np.arange(128)[None, :],
                    np.float32(0.0), np.float32(NEG)).astype(np.float32)
    in_maps = []
    for c in range(N_CORES):
        rsl = slice(LD * c, LD * (c + 1))
        wqt = np.ascontiguousarray((np.asarray(Wq)[rsl] * scale).T
                                   ).reshape(8, 128, LD).astype(np.float32)
        wkt = np.ascontiguousarray(np.asarray(Wk)[rsl].T
                                   ).reshape(8, 128, LD).astype(np.float32)
        wvt = np.ascontiguousarray(np.asarray(Wv)[rsl].T
                                   ).reshape(8, 128, LD).astype(np.float32)
        wpt = np.ascontiguousarray(np.asarray(Wp)[:, rsl].T).astype(np.float32)
        in_maps.append({"xt": XT, "wqt": wqt, "wkt": wkt, "wvt": wvt,
                        "wpt": wpt, "mask": mask})
    return in_maps


def kernel(X, Wq, Wk, Wv, Wp, bp):
    nc = get_nc()
    in_maps = make_in_maps(X, Wq, Wk, Wv, Wp)
    res = bass_utils.run_bass_kernel_spmd(nc, in_maps,
                                          core_ids=list(range(N_CORES)))
    acc = np.zeros((TOK, E), dtype=np.float64)
    for c in range(N_CORES):
        acc += res.results[c]["ypart"]
    acc += np.asarray(bp, dtype=np.float64)[None, :]
    return acc.astype(np.float32).reshape(B, S, E)
